# revision 7
# baseline (speedup 1.0000x reference)
"""Multi-head attention Trainium2 kernel (8-core SPMD, no collectives).

Sharding: 8 cores = 4 batches x 2 sequence-halves. Each core receives the
full x[b] (to compute K/V over all S keys) plus its own half of the query
rows, and produces a disjoint [NQ, D] slice of the output. K/V projection
work is duplicated across the 2 cores sharing a batch (~25% extra PE work)
in exchange for zero inter-core communication.

Per-core pipeline (all matmul inputs bf16, PSUM accumulation fp32):
  A1: V = x @ Wv               -> VP [keys, h, 65] with a ones column at 64
  A2: K^T, Q^T per head-pair   -> KT/QT [2h*64, S]
  B:  S^T[k,q] = K^T.T Q^T; P = exp(0.125*S^T); acc[q, 65] += P^T V'
      (col 64 of acc = softmax denominator); normalize by reciprocal;
      transpose back to [hv, q] -> CT
  C:  out[q,:] = CT.T @ Wo + ones.T @ bo
Softmax skips max-subtraction: scores ~ N(0,1) with |s|<~10, exp is safe.
"""

import numpy as np
import ml_dtypes

import concourse.bass as bass
import concourse.bacc as bacc
import concourse.mybir as mybir
import concourse.tile as tile
from concourse import masks

B, S, D = 4, 2048, 1024
H, DQ, DV = 16, 64, 64
P = 128
NQ = S // 2            # query rows per core
NPAIR = H // 2         # head pairs (2 heads packed on 128 partitions)
NDC = D // P           # 8 contraction chunks of D
NKC = S // P           # 16 key blocks
NQB = NQ // P          # 8 query blocks
NCORES = 8
BF16 = mybir.dt.bfloat16
F32 = mybir.dt.float32


def build_nc():
    nc = bacc.Bacc("TRN2", target_bir_lowering=False, debug=False,
                   num_devices=NCORES)

    # Host supplies partition-major layouts (see kernel() below).
    xT = nc.dram_tensor("xT", [P, NDC, S], BF16, kind="ExternalInput")
    xTq = nc.dram_tensor("xTq", [P, NDC, NQ], BF16, kind="ExternalInput")
    wk = nc.dram_tensor("wk", [NPAIR, P, NDC, P], BF16, kind="ExternalInput")
    wq = nc.dram_tensor("wq", [NPAIR, P, NDC, P], BF16, kind="ExternalInput")
    wv = nc.dram_tensor("wv", [P, NDC, H * DV], BF16, kind="ExternalInput")
    wo = nc.dram_tensor("wo", [P, NPAIR, D], BF16, kind="ExternalInput")
    bo = nc.dram_tensor("bo", [1, D], BF16, kind="ExternalInput")
    out = nc.dram_tensor("out", [NQ, D], F32, kind="ExternalOutput")

    Exp = mybir.ActivationFunctionType.Exp

    with tile.TileContext(nc) as tc:
        with (
            tc.tile_pool(name="const", bufs=1) as constp,
            tc.tile_pool(name="persist", bufs=1) as persist,
        ):
            ident = constp.tile([P, P], BF16, tag="ident")
            masks.make_identity(nc, ident[:])
            ident32 = constp.tile([P, P], F32, tag="ident32")
            masks.make_identity(nc, ident32[:])
            ones = constp.tile([1, P], BF16, tag="ones")
            nc.vector.memset(ones[:], 1.0)
            bo_sb = constp.tile([1, D], BF16, tag="bo")
            nc.sync.dma_start(bo_sb[:], bo[:])

            KT = persist.tile([P, NPAIR, S], BF16, tag="KT")
            QT = persist.tile([P, NPAIR, NQ], BF16, tag="QT")
            VP = persist.tile([P, NKC, H, DV + 1], BF16, tag="VP")
            CT = persist.tile([P, NPAIR, NQ], BF16, tag="CT")

            # ones column of V' (softmax denominator accumulator)
            nc.vector.memset(VP[:, :, :, DV:DV + 1], 1.0)

            with tc.tile_pool(name="xw", bufs=1) as xw:
                xt = xw.tile([P, NDC, S], BF16, tag="xt")
                xtq = xw.tile([P, NDC, NQ], BF16, tag="xtq")
                wvt = xw.tile([P, NDC, H * DV], BF16, tag="wv")
                for dc in range(NDC):
                    nc.sync.dma_start(xt[:, dc, :], xT[:, dc, :])
                    nc.sync.dma_start(xtq[:, dc, :], xTq[:, dc, :])
                    nc.sync.dma_start(wvt[:, dc, :], wv[:, dc, :])

                # ---- A1: V projection, all heads at once ----
                with tc.tile_pool(name="psA1", bufs=2,
                                  space=bass.MemorySpace.PSUM) as psA1:
                    for sb in range(NKC):
                        vps = psA1.tile([P, H * DV], F32, tag="v")
                        for dc in range(NDC):
                            for half in range(2):
                                nc.tensor.matmul(
                                    vps[:, half * 512:(half + 1) * 512],
                                    xt[:, dc, sb * P:(sb + 1) * P],
                                    wvt[:, dc, half * 512:(half + 1) * 512],
                                    start=(dc == 0), stop=(dc == NDC - 1))
                        nc.vector.tensor_copy(
                            VP[:, sb, :, 0:DV],
                            vps[:].rearrange("p (h v) -> p h v", h=H))

                # ---- A2: K^T and Q^T projections per head pair ----
                with (
                    tc.tile_pool(name="wkq", bufs=2) as wkq,
                    tc.tile_pool(name="psA2", bufs=6,
                                 space=bass.MemorySpace.PSUM) as psA2,
                ):
                    for pair in range(NPAIR):
                        wkp = wkq.tile([P, NDC, P], BF16, tag="wk")
                        nc.sync.dma_start(wkp[:], wk[pair])
                        wqp = wkq.tile([P, NDC, P], BF16, tag="wq")
                        nc.sync.dma_start(wqp[:], wq[pair])

                        kps = [psA2.tile([P, 512], F32, tag="kq", name=f"kps{nb}")
                               for nb in range(4)]
                        for dc in range(NDC):
                            for nb in range(4):
                                nc.tensor.matmul(
                                    kps[nb][:],
                                    wkp[:, dc, :],
                                    xt[:, dc, nb * 512:(nb + 1) * 512],
                                    start=(dc == 0), stop=(dc == NDC - 1))
                        for nb in range(4):
                            nc.vector.tensor_copy(
                                KT[:, pair, nb * 512:(nb + 1) * 512],
                                kps[nb][:])

                        qps = [psA2.tile([P, 512], F32, tag="kq", name=f"qps{nb}")
                               for nb in range(2)]
                        for dc in range(NDC):
                            for nb in range(2):
                                nc.tensor.matmul(
                                    qps[nb][:],
                                    wqp[:, dc, :],
                                    xtq[:, dc, nb * 512:(nb + 1) * 512],
                                    start=(dc == 0), stop=(dc == NDC - 1))
                        for nb in range(2):
                            nc.vector.tensor_copy(
                                QT[:, pair, nb * 512:(nb + 1) * 512],
                                qps[nb][:])

            # ---- B: attention per head ----
            with (
                tc.tile_pool(name="ptp", bufs=3) as ptp,
                tc.tile_pool(name="nrm", bufs=4) as nrmp,
                tc.tile_pool(name="ahp", bufs=2) as ahp,
                tc.tile_pool(name="psB", bufs=2,
                             space=bass.MemorySpace.PSUM) as psB,
            ):
                for h in range(H):
                    pair, hh = h // 2, (h % 2) * 64
                    # acc[g] accumulates out_h^T for queries [g*512,(g+1)*512):
                    # rows 0..63 = sum_k P[k,q] V[k,v]; row 64 = sum_k P[k,q]
                    accs = [psB.tile([DV + 1, 512], F32, tag="acc",
                                     name=f"acc{g}") for g in range(2)]
                    for kc in range(NKC):
                        sc = psB.tile([P, NQ], F32, tag="sc")
                        for half in range(2):
                            nc.tensor.matmul(
                                sc[:, half * 512:(half + 1) * 512],
                                KT[hh:hh + 64, pair, kc * P:(kc + 1) * P],
                                QT[hh:hh + 64, pair,
                                   half * 512:(half + 1) * 512],
                                start=True, stop=True)
                        pt = ptp.tile([P, NQ], BF16, tag="pt")
                        nc.scalar.activation(pt[:], sc[:], Exp, scale=0.125)
                        for g in range(2):
                            nc.tensor.matmul(
                                accs[g][:],
                                VP[:, kc, h, :],
                                pt[:, g * 512:(g + 1) * 512],
                                start=(kc == 0), stop=(kc == NKC - 1))
                    for g in range(2):
                        ah = ahp.tile([DV + 1, 512], F32, tag="ah")
                        nc.vector.tensor_copy(ah[:], accs[g][:])
                        for j in range(4):
                            qb = g * 4 + j
                            tp = psB.tile([P, DV + 1], F32, tag="tp")
                            nc.tensor.transpose(
                                tp[:], ah[:, j * P:(j + 1) * P],
                                ident32[0:DV + 1, 0:DV + 1])
                            rec = nrmp.tile([P, 1], F32, tag="rec")
                            nc.vector.reciprocal(rec[:], tp[:, DV:DV + 1])
                            nrm = nrmp.tile([P, DV], BF16, tag="nrm")
                            nc.vector.tensor_scalar_mul(
                                nrm[:], tp[:, 0:DV], rec[:])
                            tr = psB.tile([DV, P], BF16, tag="tp", name="tr")
                            nc.tensor.transpose(tr[:], nrm[:], ident[:])
                            nc.vector.tensor_copy(
                                CT[hh:hh + 64, pair, qb * P:(qb + 1) * P],
                                tr[:])

            # ---- C: output projection + bias ----
            with (
                tc.tile_pool(name="wop", bufs=1) as wop,
                tc.tile_pool(name="outp", bufs=2) as outp,
                tc.tile_pool(name="psC", bufs=2,
                             space=bass.MemorySpace.PSUM) as psC,
            ):
                wot = wop.tile([P, NPAIR, D], BF16, tag="wo")
                for pc in range(NPAIR):
                    nc.sync.dma_start(wot[:, pc, :], wo[:, pc, :])
                for qb in range(NQB):
                    ops = psC.tile([P, D], F32, tag="o")
                    for pc in range(NPAIR):
                        for half in range(2):
                            nc.tensor.matmul(
                                ops[:, half * 512:(half + 1) * 512],
                                CT[:, pc, qb * P:(qb + 1) * P],
                                wot[:, pc, half * 512:(half + 1) * 512],
                                start=(pc == 0), stop=False)
                    for half in range(2):
                        nc.tensor.matmul(
                            ops[:, half * 512:(half + 1) * 512],
                            ones[:],
                            bo_sb[:, half * 512:(half + 1) * 512],
                            start=False, stop=True)
                    outsb = outp.tile([P, D], F32, tag="out")
                    nc.vector.tensor_copy(outsb[:], ops[:])
                    nc.sync.dma_start(out[qb * P:(qb + 1) * P, :], outsb[:])

    nc.compile()
    return nc


def make_in_maps(x, Wq, Wk, Wv, Wo, bo):
    bf = ml_dtypes.bfloat16
    x = np.asarray(x, np.float32)

    def pm(a):  # [D, N] -> partition-major [P, NDC, N]
        return np.ascontiguousarray(
            a.reshape(NDC, P, a.shape[1]).transpose(1, 0, 2)).astype(bf)

    def wpairs(W):  # [H, D, 64] -> [NPAIR, P, NDC, P]
        a = np.asarray(W, np.float32).transpose(1, 0, 2).reshape(D, H * 64)
        return np.ascontiguousarray(
            a.reshape(NDC, P, NPAIR, P).transpose(2, 1, 0, 3)).astype(bf)

    wk_h = wpairs(Wk)
    wq_h = wpairs(Wq)
    wv_h = pm(np.asarray(Wv, np.float32).transpose(1, 0, 2).reshape(D, H * DV))
    wo_h = np.ascontiguousarray(
        np.asarray(Wo, np.float32).reshape(NPAIR, P, D)
        .transpose(1, 0, 2)).astype(bf)
    bo_h = np.asarray(bo, np.float32).reshape(1, D).astype(bf)

    in_maps = []
    for c in range(NCORES):
        b, qs = c // 2, (c % 2) * NQ
        in_maps.append({
            "xT": pm(np.ascontiguousarray(x[b].T)),
            "xTq": pm(np.ascontiguousarray(x[b, qs:qs + NQ].T)),
            "wk": wk_h, "wq": wq_h, "wv": wv_h, "wo": wo_h, "bo": bo_h,
        })
    return in_maps


def kernel(x, Wq, Wk, Wv, Wo, bo):
    from concourse.bass_utils import run_bass_kernel_spmd
    in_maps = make_in_maps(x, Wq, Wk, Wv, Wo, bo)
    nc = build_nc()
    res = run_bass_kernel_spmd(nc, in_maps, list(range(NCORES))).results
    full = np.empty((B, S, D), np.float32)
    for c in range(NCORES):
        b, qs = c // 2, (c % 2) * NQ
        full[b, qs:qs + NQ, :] = np.asarray(res[c]["out"], np.float32)
    return full
